# revision 1
# baseline (speedup 1.0000x reference)
"""Trainium2 Bass kernel for the BlockDiagonalACDC layer.

reference:  y = riffle(idct2(gconv(dct2(gconv(x, A)), D))) + bias

Every op is linear along the feature dim (d=4096), so the whole layer is
    out = x @ W_A @ Ct @ W_D @ (Gm P_riffle) + bias
with W_A = blockdiag(A_g^T), W_D = blockdiag(D_g^T) (runtime inputs), and
Ct (unnormalized DCT-II) / Gp = Gm @ P_riffle (inverse DCT-II, columns
riffled) compile-time constants.

Sharding: pure data parallel — batch 16384 split as 2048 rows per core
across 8 cores; A/D/bias/DCT-constants replicated. No communication.

On-device pipeline per core, in feature-transposed space (feature on
partitions, batch on the free dim), processed in 4 chunks of 512 rows:
  T:  x[n,f] --cast bf16 + PE-transpose--> xT[f,n]
  A:  z1T = blockdiag(A^T)^T-apply (32 grouped matmuls)
  B:  z2T[k,n] = sum_f Ct[f,k] z1T[f,n]   (dense, Ct streamed as lhsT)
  C:  z3T = grouped matmuls with D
  D:  out[n,j] = sum_k z3T[k,n] Gp[k,j] + bias  (operands swapped so the
      output lands un-transposed; bias folded in as a K=1 matmul)
"""

import numpy as np
import ml_dtypes

import concourse.bacc as bacc
import concourse.mybir as mybir
from concourse.tile import TileContext
from concourse.bass_utils import run_bass_kernel_spmd
from concourse.masks import make_identity

N_BATCH, D_FEAT, GROUPS = 16384, 4096, 32
N_CORES = 8
N_SHARD = N_BATCH // N_CORES      # 2048 rows per core
CHUNK = 512                       # batch rows processed per pipeline chunk
N_CHUNKS = N_SHARD // CHUNK       # 4
FTILES = D_FEAT // 128            # 32 feature partition-tiles
JC = 256                          # output-feature width per pass-D strip
N_JC = D_FEAT // JC               # 16

_BF16 = mybir.dt.bfloat16
_F32 = mybir.dt.float32


def _host_constants():
    """DCT-II matrix Ct, inverse Gp (riffled), pre-swizzled for SBUF tiles."""
    N = D_FEAT
    j = np.arange(N, dtype=np.float64)
    k = np.arange(N, dtype=np.float64)[:, None]
    ang = np.pi * k * (2.0 * j[None, :] + 1.0) / (2.0 * N)
    C = 2.0 * np.cos(ang)               # dct2(y) = y @ C.T
    Ct = np.ascontiguousarray(C.T)      # [f, k]
    w = np.ones(N); w[0] = 0.5
    Gm = (1.0 / N) * w[:, None] * np.cos(ang)   # idct2(w) = w @ Gm
    half = N // 2
    perm = np.empty(N, dtype=np.int64)
    for c in range(2):
        perm[c * half:(c + 1) * half] = 2 * np.arange(half) + c
    Gp = np.ascontiguousarray(Gm[:, perm])      # [k, j]

    # ct_host[kt, p, fc, kk] = Ct[fc*128 + p, kt*128 + kk]  (1 MiB per kt strip)
    ct_host = np.ascontiguousarray(
        Ct.reshape(FTILES, 128, FTILES, 128).transpose(2, 1, 0, 3)
    ).astype(ml_dtypes.bfloat16)
    # gp_host[jc, p, kc, jj] = Gp[kc*128 + p, jc*JC + jj]
    gp_host = np.ascontiguousarray(
        Gp.reshape(FTILES, 128, N_JC, JC).transpose(2, 1, 0, 3)
    ).astype(ml_dtypes.bfloat16)
    return ct_host, gp_host


def _build_program():
    nc = bacc.Bacc()
    xs = nc.dram_tensor("xs", (N_SHARD, D_FEAT), _F32, kind="ExternalInput")
    Aw = nc.dram_tensor("Aw", (GROUPS, 128, 128), _F32, kind="ExternalInput")
    Dw = nc.dram_tensor("Dw", (GROUPS, 128, 128), _F32, kind="ExternalInput")
    bias = nc.dram_tensor("bias", (1, D_FEAT), _F32, kind="ExternalInput")
    ct = nc.dram_tensor("ct", (FTILES, 128, FTILES, 128), _BF16, kind="ExternalInput")
    gp = nc.dram_tensor("gp", (N_JC, 128, FTILES, JC), _BF16, kind="ExternalInput")
    out = nc.dram_tensor("out", (N_SHARD, D_FEAT), _F32, kind="ExternalOutput")

    with TileContext(nc) as tc:
        with (
            tc.tile_pool(name="const", bufs=1) as constp,
            tc.tile_pool(name="stage", bufs=3) as stagep,
            tc.tile_pool(name="xbf", bufs=4) as xbfp,
            tc.tile_pool(name="ctp", bufs=2) as ctp,
            tc.tile_pool(name="gpp", bufs=2) as gpp,
            tc.tile_pool(name="ost", bufs=4) as ostp,
            tc.tile_pool(name="mm_ps", bufs=4, space="PSUM") as mmp,
            tc.tile_pool(name="tp_ps", bufs=2, space="PSUM") as tpp,
            tc.tile_pool(name="pd_ps", bufs=2, space="PSUM") as pdp,
        ):
            ident = constp.tile([128, 128], _BF16, tag="ident")
            make_identity(nc, ident[:])

            ones1 = constp.tile([1, 128], _BF16, tag="ones1")
            nc.gpsimd.memset(ones1[:], 1.0)

            bias_bf = constp.tile([1, D_FEAT], _BF16, tag="bias")
            nc.gpsimd.dma_start(bias_bf[:], bias[:])  # casts f32 -> bf16

            # ---- prep: AT = per-group transpose of A (lhsT for the gconv), same for D
            AT = constp.tile([128, D_FEAT], _BF16, tag="AT")
            DT = constp.tile([128, D_FEAT], _BF16, tag="DT")
            for (W, T) in ((Aw, AT), (Dw, DT)):
                wbf = stagep.tile([128, D_FEAT], _BF16, tag="stage")
                for g in range(GROUPS):
                    nc.gpsimd.dma_start(wbf[:, g * 128:(g + 1) * 128], W[g])
                for g4 in range(GROUPS // 4):
                    ps = tpp.tile([128, 512], _BF16, tag="tp")
                    for gg in range(4):
                        g = g4 * 4 + gg
                        nc.tensor.transpose(
                            ps[:, gg * 128:(gg + 1) * 128],
                            wbf[:, g * 128:(g + 1) * 128],
                            ident[:],
                        )
                    nc.vector.tensor_copy(T[:, g4 * 512:(g4 + 1) * 512], ps[:])

            for ci in range(N_CHUNKS):
                r0 = ci * CHUNK
                # ---- T: load+cast 4 x-row-tiles, transpose into xT[f, n]
                xT = stagep.tile([128, FTILES * CHUNK], _BF16, tag="stage")
                xbfs = []
                for nt in range(CHUNK // 128):
                    xbf = xbfp.tile([128, D_FEAT], _BF16, tag="xbf")
                    nc.gpsimd.dma_start(xbf[:], xs[r0 + nt * 128: r0 + (nt + 1) * 128, :])
                    xbfs.append(xbf)
                for fc in range(FTILES):
                    ps = tpp.tile([128, 512], _BF16, tag="tp")
                    for nt in range(CHUNK // 128):
                        nc.tensor.transpose(
                            ps[:, nt * 128:(nt + 1) * 128],
                            xbfs[nt][:, fc * 128:(fc + 1) * 128],
                            ident[:],
                        )
                    nc.vector.tensor_copy(xT[:, fc * CHUNK:(fc + 1) * CHUNK], ps[:])

                # ---- A: z1T[g-block] = AT[g].T @ xT[g-block]
                z1 = stagep.tile([128, FTILES * CHUNK], _BF16, tag="stage")
                for g in range(GROUPS):
                    ps = mmp.tile([128, CHUNK], _F32, tag="mm")
                    nc.tensor.matmul(
                        ps[:], AT[:, g * 128:(g + 1) * 128],
                        xT[:, g * CHUNK:(g + 1) * CHUNK],
                        start=True, stop=True,
                    )
                    nc.scalar.copy(z1[:, g * CHUNK:(g + 1) * CHUNK], ps[:])

                # ---- B: z2T[kt-block] = sum_f ct[f, kt-block].T-apply
                z2 = stagep.tile([128, FTILES * CHUNK], _BF16, tag="stage")
                for kt in range(FTILES):
                    ctt = ctp.tile([128, FTILES * 128], _BF16, tag="ct")
                    nc.sync.dma_start(ctt[:], ct[kt])
                    ps = mmp.tile([128, CHUNK], _F32, tag="mm")
                    for fc in range(FTILES):
                        nc.tensor.matmul(
                            ps[:], ctt[:, fc * 128:(fc + 1) * 128],
                            z1[:, fc * CHUNK:(fc + 1) * CHUNK],
                            start=(fc == 0), stop=(fc == FTILES - 1),
                        )
                    nc.vector.tensor_copy(z2[:, kt * CHUNK:(kt + 1) * CHUNK], ps[:])

                # ---- C: z3T[g-block] = DT[g].T @ z2T[g-block]
                z3 = stagep.tile([128, FTILES * CHUNK], _BF16, tag="stage")
                for g in range(GROUPS):
                    ps = mmp.tile([128, CHUNK], _F32, tag="mm")
                    nc.tensor.matmul(
                        ps[:], DT[:, g * 128:(g + 1) * 128],
                        z2[:, g * CHUNK:(g + 1) * CHUNK],
                        start=True, stop=True,
                    )
                    nc.scalar.copy(z3[:, g * CHUNK:(g + 1) * CHUNK], ps[:])

                # ---- D: out[n, j] = sum_k z3T[k, n] Gp[k, j] + bias[j]
                for jc in range(N_JC):
                    gpt = gpp.tile([128, FTILES * JC], _BF16, tag="gp")
                    nc.sync.dma_start(gpt[:], gp[jc])
                    for nt in range(CHUNK // 128):
                        ps = pdp.tile([128, JC], _F32, tag="pd")
                        for kc in range(FTILES):
                            nc.tensor.matmul(
                                ps[:],
                                z3[:, kc * CHUNK + nt * 128: kc * CHUNK + (nt + 1) * 128],
                                gpt[:, kc * JC:(kc + 1) * JC],
                                start=(kc == 0), stop=False,
                            )
                        nc.tensor.matmul(
                            ps[:], ones1[:], bias_bf[0:1, jc * JC:(jc + 1) * JC],
                            start=False, stop=True,
                        )
                        ost = ostp.tile([128, JC], _F32, tag="ost")
                        nc.scalar.copy(ost[:], ps[:])
                        nc.sync.dma_start(
                            out[r0 + nt * 128: r0 + (nt + 1) * 128,
                                jc * JC:(jc + 1) * JC],
                            ost[:],
                        )
    nc.finalize()
    return nc


_CACHE = {}


def kernel(x, A, D, bias):
    if "nc" not in _CACHE:
        _CACHE["nc"] = _build_program()
        _CACHE["consts"] = _host_constants()
    nc = _CACHE["nc"]
    ct_host, gp_host = _CACHE["consts"]

    x = np.ascontiguousarray(x, dtype=np.float32)
    in_maps = []
    for c in range(N_CORES):
        in_maps.append({
            "xs": x[c * N_SHARD:(c + 1) * N_SHARD],
            "Aw": np.ascontiguousarray(A, dtype=np.float32),
            "Dw": np.ascontiguousarray(D, dtype=np.float32),
            "bias": np.ascontiguousarray(bias, dtype=np.float32),
            "ct": ct_host,
            "gp": gp_host,
        })
    res = run_bass_kernel_spmd(nc, in_maps, core_ids=list(range(N_CORES)))
    return np.concatenate([res.results[c]["out"] for c in range(N_CORES)], axis=0)


# revision 4
# speedup vs baseline: 9005.0276x; 9005.0276x over previous
"""Trainium2 Bass kernel for the BlockDiagonalACDC layer.

reference:  y = riffle(idct2(gconv(dct2(gconv(x, A)), D))) + bias

Every op is linear along the feature dim (d=4096), so the whole layer is
    out = x @ W_A @ Ct @ W_D @ (Gm P_riffle) + bias
with W_A = blockdiag(A_g^T), W_D = blockdiag(D_g^T) (runtime inputs), and
Ct (unnormalized DCT-II) / Gp = Gm @ P_riffle (inverse DCT-II, columns
riffled) compile-time constants.

Sharding: pure data parallel — batch 16384 split as 2048 rows per core
across 8 cores; A/D/bias/DCT-constants replicated. No communication.

On-device pipeline per core, in feature-transposed space (feature on
partitions, batch on the free dim), processed in 4 chunks of 512 rows:
  T:  x[n,f] --cast bf16 + PE-transpose--> xT[f,n]
  A:  z1T = blockdiag(A^T)^T-apply (32 grouped matmuls)
  B:  z2T[k,n] = sum_f Ct[f,k] z1T[f,n]   (dense, Ct streamed as lhsT)
  C:  z3T = grouped matmuls with D
  D:  out[n,j] = sum_k z3T[k,n] Gp[k,j] + bias  (operands swapped so the
      output lands un-transposed; bias folded in as a K=1 matmul)
"""

import numpy as np
import ml_dtypes

import concourse.bacc as bacc
import concourse.mybir as mybir
from concourse.tile import TileContext
from concourse.bass_utils import run_bass_kernel_spmd
from concourse.masks import make_identity

N_BATCH, D_FEAT, GROUPS = 16384, 4096, 32
N_CORES = 8
N_SHARD = N_BATCH // N_CORES      # 2048 rows per core
CHUNK = 512                       # batch rows processed per pipeline chunk
N_CHUNKS = N_SHARD // CHUNK       # 4
FTILES = D_FEAT // 128            # 32 feature partition-tiles
JC = 256                          # output-feature width per pass-D strip
N_JC = D_FEAT // JC               # 16

_BF16 = mybir.dt.bfloat16
_F32 = mybir.dt.float32


def _host_constants():
    """DCT-II matrix Ct, inverse Gp (riffled), pre-swizzled for SBUF tiles."""
    N = D_FEAT
    j = np.arange(N, dtype=np.float64)
    k = np.arange(N, dtype=np.float64)[:, None]
    ang = np.pi * k * (2.0 * j[None, :] + 1.0) / (2.0 * N)
    C = 2.0 * np.cos(ang)               # dct2(y) = y @ C.T
    Ct = np.ascontiguousarray(C.T)      # [f, k]
    w = np.ones(N); w[0] = 0.5
    Gm = (1.0 / N) * w[:, None] * np.cos(ang)   # idct2(w) = w @ Gm
    half = N // 2
    perm = np.empty(N, dtype=np.int64)
    for c in range(2):
        perm[c * half:(c + 1) * half] = 2 * np.arange(half) + c
    Gp = np.ascontiguousarray(Gm[:, perm])      # [k, j]

    # ct_host[kt, p, fc, kk] = Ct[fc*128 + p, kt*128 + kk]  (1 MiB per kt strip)
    ct_host = np.ascontiguousarray(
        Ct.reshape(FTILES, 128, FTILES, 128).transpose(2, 1, 0, 3)
    ).astype(ml_dtypes.bfloat16)
    # gp_host[jc, p, kc, jj] = Gp[kc*128 + p, jc*JC + jj]
    gp_host = np.ascontiguousarray(
        Gp.reshape(FTILES, 128, N_JC, JC).transpose(2, 1, 0, 3)
    ).astype(ml_dtypes.bfloat16)
    return ct_host, gp_host


def _build_program(reps=1):
    nc = bacc.Bacc()
    xs = nc.dram_tensor("xs", (N_SHARD, D_FEAT), _F32, kind="ExternalInput")
    Aw = nc.dram_tensor("Aw", (GROUPS, 128, 128), _F32, kind="ExternalInput")
    Dw = nc.dram_tensor("Dw", (GROUPS, 128, 128), _F32, kind="ExternalInput")
    bias = nc.dram_tensor("bias", (1, D_FEAT), _F32, kind="ExternalInput")
    ct = nc.dram_tensor("ct", (FTILES, 128, FTILES, 128), _BF16, kind="ExternalInput")
    gp = nc.dram_tensor("gp", (N_JC, 128, FTILES, JC), _BF16, kind="ExternalInput")
    out = nc.dram_tensor("out", (N_SHARD, D_FEAT), _F32, kind="ExternalOutput")

    with TileContext(nc) as tc:
        with (
            tc.tile_pool(name="const", bufs=1) as constp,
            tc.tile_pool(name="stage", bufs=3) as stagep,
            tc.tile_pool(name="xbf", bufs=4) as xbfp,
            tc.tile_pool(name="ctp", bufs=2) as ctp,
            tc.tile_pool(name="gpp", bufs=2) as gpp,
            tc.tile_pool(name="ost", bufs=4) as ostp,
            tc.tile_pool(name="mm_ps", bufs=4, space="PSUM") as mmp,
            tc.tile_pool(name="tp_ps", bufs=2, space="PSUM") as tpp,
            tc.tile_pool(name="pd_ps", bufs=2, space="PSUM") as pdp,
        ):
            ident = constp.tile([128, 128], _BF16, tag="ident")
            make_identity(nc, ident[:])

            ones1 = constp.tile([1, 128], _BF16, tag="ones1")
            nc.gpsimd.memset(ones1[:], 1.0)

            bias_bf = constp.tile([1, D_FEAT], _BF16, tag="bias")
            nc.gpsimd.dma_start(bias_bf[:], bias[:])  # casts f32 -> bf16

            # ---- prep: AT = per-group transpose of A (lhsT for the gconv), same for D
            AT = constp.tile([128, D_FEAT], _BF16, tag="AT")
            DT = constp.tile([128, D_FEAT], _BF16, tag="DT")
            for (W, T) in ((Aw, AT), (Dw, DT)):
                wbf = stagep.tile([128, D_FEAT], _BF16, tag="stage")
                for g in range(GROUPS):
                    nc.gpsimd.dma_start(wbf[:, g * 128:(g + 1) * 128], W[g])
                for g4 in range(GROUPS // 4):
                    ps = tpp.tile([128, 512], _BF16, tag="tp")
                    for gg in range(4):
                        g = g4 * 4 + gg
                        nc.tensor.transpose(
                            ps[:, gg * 128:(gg + 1) * 128],
                            wbf[:, g * 128:(g + 1) * 128],
                            ident[:],
                        )
                    nc.vector.tensor_copy(T[:, g4 * 512:(g4 + 1) * 512], ps[:])

            rep_ctx = tc.For_i(0, reps, 1) if reps > 1 else None
            if rep_ctx is not None:
                rep_ctx.__enter__()
            for ci in range(N_CHUNKS):
                r0 = ci * CHUNK
                # ---- T: load+cast 4 x-row-tiles, transpose into xT[f, n]
                xT = stagep.tile([128, FTILES * CHUNK], _BF16, tag="stage")
                xbfs = []
                for nt in range(CHUNK // 128):
                    xbf = xbfp.tile([128, D_FEAT], _BF16, tag="xbf")
                    nc.gpsimd.dma_start(xbf[:], xs[r0 + nt * 128: r0 + (nt + 1) * 128, :])
                    xbfs.append(xbf)
                for fc in range(FTILES):
                    ps = tpp.tile([128, 512], _BF16, tag="tp")
                    for nt in range(CHUNK // 128):
                        nc.tensor.transpose(
                            ps[:, nt * 128:(nt + 1) * 128],
                            xbfs[nt][:, fc * 128:(fc + 1) * 128],
                            ident[:],
                        )
                    nc.vector.tensor_copy(xT[:, fc * CHUNK:(fc + 1) * CHUNK], ps[:])

                # ---- A: z1T[g-block] = AT[g].T @ xT[g-block]
                z1 = stagep.tile([128, FTILES * CHUNK], _BF16, tag="stage")
                for g in range(GROUPS):
                    ps = mmp.tile([128, CHUNK], _F32, tag="mm")
                    nc.tensor.matmul(
                        ps[:], AT[:, g * 128:(g + 1) * 128],
                        xT[:, g * CHUNK:(g + 1) * CHUNK],
                        start=True, stop=True,
                    )
                    nc.scalar.copy(z1[:, g * CHUNK:(g + 1) * CHUNK], ps[:])

                # ---- B: z2T[kt-block] = sum_f ct[f, kt-block].T-apply
                z2 = stagep.tile([128, FTILES * CHUNK], _BF16, tag="stage")
                for kt in range(FTILES):
                    ctt = ctp.tile([128, FTILES * 128], _BF16, tag="ct")
                    nc.sync.dma_start(ctt[:], ct[kt])
                    ps = mmp.tile([128, CHUNK], _F32, tag="mm")
                    for fc in range(FTILES):
                        nc.tensor.matmul(
                            ps[:], ctt[:, fc * 128:(fc + 1) * 128],
                            z1[:, fc * CHUNK:(fc + 1) * CHUNK],
                            start=(fc == 0), stop=(fc == FTILES - 1),
                        )
                    nc.vector.tensor_copy(z2[:, kt * CHUNK:(kt + 1) * CHUNK], ps[:])

                # ---- C: z3T[g-block] = DT[g].T @ z2T[g-block]
                z3 = stagep.tile([128, FTILES * CHUNK], _BF16, tag="stage")
                for g in range(GROUPS):
                    ps = mmp.tile([128, CHUNK], _F32, tag="mm")
                    nc.tensor.matmul(
                        ps[:], DT[:, g * 128:(g + 1) * 128],
                        z2[:, g * CHUNK:(g + 1) * CHUNK],
                        start=True, stop=True,
                    )
                    nc.scalar.copy(z3[:, g * CHUNK:(g + 1) * CHUNK], ps[:])

                # ---- D: out[n, j] = sum_k z3T[k, n] Gp[k, j] + bias[j]
                for jc in range(N_JC):
                    gpt = gpp.tile([128, FTILES * JC], _BF16, tag="gp")
                    nc.sync.dma_start(gpt[:], gp[jc])
                    for nt in range(CHUNK // 128):
                        ps = pdp.tile([128, JC], _F32, tag="pd")
                        for kc in range(FTILES):
                            nc.tensor.matmul(
                                ps[:],
                                z3[:, kc * CHUNK + nt * 128: kc * CHUNK + (nt + 1) * 128],
                                gpt[:, kc * JC:(kc + 1) * JC],
                                start=(kc == 0), stop=False,
                            )
                        nc.tensor.matmul(
                            ps[:], ones1[:], bias_bf[0:1, jc * JC:(jc + 1) * JC],
                            start=False, stop=True,
                        )
                        ost = ostp.tile([128, JC], _F32, tag="ost")
                        nc.scalar.copy(ost[:], ps[:])
                        nc.sync.dma_start(
                            out[r0 + nt * 128: r0 + (nt + 1) * 128,
                                jc * JC:(jc + 1) * JC],
                            ost[:],
                        )
            if rep_ctx is not None:
                rep_ctx.__exit__(None, None, None)
    nc.finalize()
    return nc


_CACHE = {}


def kernel(x, A, D, bias):
    if "nc" not in _CACHE:
        _CACHE["nc"] = _build_program()
        _CACHE["consts"] = _host_constants()
    nc = _CACHE["nc"]
    ct_host, gp_host = _CACHE["consts"]

    x = np.ascontiguousarray(x, dtype=np.float32)
    in_maps = []
    for c in range(N_CORES):
        in_maps.append({
            "xs": x[c * N_SHARD:(c + 1) * N_SHARD],
            "Aw": np.ascontiguousarray(A, dtype=np.float32),
            "Dw": np.ascontiguousarray(D, dtype=np.float32),
            "bias": np.ascontiguousarray(bias, dtype=np.float32),
            "ct": ct_host,
            "gp": gp_host,
        })
    res = run_bass_kernel_spmd(nc, in_maps, core_ids=list(range(N_CORES)))
    return np.concatenate([res.results[c]["out"] for c in range(N_CORES)], axis=0)
